# revision 15
# baseline (speedup 1.0000x reference)
"""Trainium2 Bass kernel for DiceLoss (hard-argmax dice, ignore background, mean).

Problem (hardcoded shapes):
  y_true: [16, 512, 512] int32 in [0, 8)
  y_pred: [16, 8, 512, 512] float32
  out   : scalar float32 = mean over classes 1..7 of
          (2*tp + eps) / (2*tp + fp + fn + eps)
  with pred_cls = argmax_c y_pred, one-hot tp/fp/fn sums over all pixels.

Strategy (8 NeuronCores, data-parallel over batch):
  - Each core processes 2 of the 16 batch images (SPMD, same NEFF), streamed
    as 5 chunks of [512, 1024, 1024, 1024, 512] pixel-columns (the middle
    chunk spans the image boundary -- pixels are pixels for global tp/fp/fn
    sums; the small tail chunk shortens the end-of-stream compute that
    cannot overlap DMA).
  - y_pred is cast f32 -> fp16 during the DMA itself (SWDGE CME cast, one
    8-channel DMA per chunk; HBM read traffic unchanged).  fp16
    equality-vs-max introduces spurious argmax ties at ~5e-4 of pixels ->
    rel err ~2e-4 on the final dice (tolerance 2e-2).
  - Labels are staged as uint8 (lossless re-encoding of values 0..7),
    loaded upfront via HWDGE and converted once to fp16 on the
    otherwise-idle ScalarE.  Each chunk's gt masks are computed one chunk
    AHEAD (double-buffered G tile), so the per-chunk DVE work on the
    critical path (max tree + pred masks) stays under the chunk's DMA
    time and the final chunk leaves only ~4us of compute after the last
    HBM byte.
  - DVE (VectorE), all ops in measured fast perf modes (accum_out is never
    used: it drops DVE to 1x mode on HW; scalar_tensor_tensor is 1x-only):
      * 7-op pairwise tensor_tensor MAX tree (fp16, 2x)
      * pred masks (ch[c] == m): one batched 4D-AP tensor_tensor IS_EQUAL
        over all 7 classes (fp16, 2x)
      * gt masks (labels == c): 7 tensor_scalar IS_EQUAL (fp16, 4x)
      * 2 batched tail-compare ops per chunk
  - Mask layout: per class, 10 subtiles of 128 columns = [127 px | 1 ones
    col]; a 1024-px chunk = 8 full subtiles + an 8-px tail in s=8, a
    512-px chunk = 4 full + a 4-px tail (s=4 for the head chunk, virgin
    s=9 for the last chunk so no pad re-zeroing is ever needed).
  - TensorE: per (class, subtile) one matmul psum_c += P_cs^T @ G_cs
    (N=128) accumulated over all subtiles/chunks.  In the [128,128] psum:
    diag[0:127] = tp, col 127 = per-col pred counts, row 127 = per-col gt
    counts -- all three statistics from the same matmul stream.  One psum
    bank per class so the final drains overlap the tail chunk's compute.
  - Host: sums the 8 cores' exact-integer partials and forms the dice mean
    in float32, mirroring the reference arithmetic.
"""

import numpy as np

EPS = 1e-05

# Problem geometry (hardcoded per the harness contract).
N_CORES = 8
NB = 2             # batch images per core
C = 8              # classes
P = 128            # SBUF partitions
FP = 2048          # free-dim elements per image plane ([128, 2048] = 512*512)
FT = NB * FP       # label columns per core (4096)
FMAX = 1024        # max pixels per chunk (class stride inside chall)
NSUBT = 10         # subtiles per class block (incl. virgin tail slot s=9)
BW = NSUBT * 128   # mask-tile columns per class block (1280)

# Chunk schedule: (pieces, px, tail_slot); pieces = [(img, off, len), ...].
# Global label offset of a chunk = img*FP + off of its first piece.
CHUNKS = [
    ([(0, 0, 512)], 512, 4),
    ([(0, 512, 1024)], 1024, 8),
    ([(0, 1536, 512), (1, 0, 512)], 1024, 8),
    ([(1, 512, 1024)], 1024, 8),
    ([(1, 1536, 512)], 512, 9),
]

_CACHED_NC = None


def build_bass():
    """Build the Bass kernel (same NEFF for all 8 cores)."""
    from contextlib import ExitStack

    import concourse.bacc as bacc
    import concourse.tile as tile
    from concourse import mybir

    nc = bacc.Bacc(None, target_bir_lowering=False)

    yp = nc.dram_tensor("yp", [NB, C, P, FP], mybir.dt.float32, kind="ExternalInput")
    yt = nc.dram_tensor("yt", [NB, P, FP], mybir.dt.uint8, kind="ExternalInput")
    # per class: [128, 128] psum (diag = tp, col 127 = pred cnt, row 127 = gt cnt)
    tp_out = nc.dram_tensor("tp_out", [7, P, 128], mybir.dt.float32, kind="ExternalOutput")

    with tile.TileContext(nc) as tc, ExitStack() as ctx:
        chpool = ctx.enter_context(tc.tile_pool(name="ch", bufs=3))
        mpool = ctx.enter_context(tc.tile_pool(name="mx", bufs=2))
        mtmp = ctx.enter_context(tc.tile_pool(name="mtmp", bufs=2))
        maskp = ctx.enter_context(tc.tile_pool(name="mask", bufs=1))
        abspool = ctx.enter_context(tc.tile_pool(name="ab", bufs=2))
        drainp = ctx.enter_context(tc.tile_pool(name="drain", bufs=1))
        psump = ctx.enter_context(tc.tile_pool(name="psum", bufs=1, space="PSUM"))

        # Persistent tiles.  G is manually double-buffered (G0/G1) so chunk
        # k+1's gt masks can be built while chunk k's matmuls read G[k%2].
        Pm = maskp.tile([P, 7 * BW], mybir.dt.float16, name="Pm", tag="Pm")
        G01 = [
            maskp.tile([P, 7 * BW], mybir.dt.float16, name=f"G{i}", tag=f"G{i}")
            for i in range(2)
        ]
        # cvec[:, i] = i+1 (class constants for the batched gt tail compare)
        cvec = maskp.tile([P, 7], mybir.dt.float16, name="cvec", tag="cvec")
        # all labels upfront: uint8 via HWDGE, one fp16 convert on ScalarE
        t8 = maskp.tile([P, FT], mybir.dt.uint8, name="t8", tag="t8")
        tfall = maskp.tile([P, FT], mybir.dt.float16, name="tfall", tag="tfall")
        for n in range(NB):
            nc.sync.dma_start(out=t8[:, n * FP : (n + 1) * FP], in_=yt[n])
        nc.scalar.copy(out=tfall, in_=t8)

        # One-time init (column-targeted; a full-tile memset costs ~7us of
        # DVE and would delay the first chunk's compute):
        #   - ones column (col 127 of each subtile block)
        #   - zero pads of the tail subtiles (s=4/s=9 cols[4:127], s=8
        #     cols[8:127]); mask ops only ever write px columns, so pads
        #     stay 0 and ones columns stay 1 across all chunks.
        # ones cols: write a 16-wide stripe [112:128] (fast mode; cols
        # 112..126 of full subtiles get overwritten by px writes, and the
        # pad memsets below re-zero them inside tail subtiles)
        for t in (Pm, G01[0], G01[1]):
            blocks = t[:, :].rearrange("p (c s w) -> p c s w", c=7, w=128)
            nc.vector.memset(blocks[:, :, :, 112:128], 1.0)
            nc.vector.memset(blocks[:, :, 4, 4:127], 0.0)
            nc.vector.memset(blocks[:, :, 8, 8:127], 0.0)
            nc.vector.memset(blocks[:, :, 9, 4:127], 0.0)
        for i in range(7):
            nc.vector.memset(cvec[:, i : i + 1], float(i + 1))
        # bias constants for the ACT gt-mask ops: col 0 = +1.0 (relu bias),
        # col c = -c (abs bias)
        bvec = maskp.tile([P, 8], mybir.dt.float16, name="bvec", tag="bvec")
        nc.vector.memset(bvec[:, 0:1], 1.0)
        for c in range(1, 8):
            nc.vector.memset(bvec[:, c : c + 1], -float(c))

        # One psum bank per class: drains only wait on their own chain.
        psums = [
            psump.tile([P, 128], mybir.dt.float32, name=f"ps{c}", tag=f"ps{c}")
            for c in range(7)
        ]

        def emit_gt(k):
            """gt masks for chunk k.  Main masks run on the otherwise-idle
            ScalarE as relu(1 - |label - c|) -- exact for integer labels in
            fp16 -- keeping DVE free for the max tree + pred compares.  The
            tiny tail compare stays on DVE (1 batched op)."""
            pieces, fk, tail_slot = CHUNKS[k]
            n_full = fk // 127
            tail = fk - 127 * n_full
            main = 127 * n_full
            g0 = pieces[0][0] * FP + pieces[0][1]  # global label offset
            g_blocks = G01[k % 2][:, :].rearrange("p (c s w) -> p c s w", c=7, w=128)
            for c in range(1, C):
                ab = abspool.tile([P, FMAX], mybir.dt.float16, name="ab", tag="ab")
                nc.scalar.activation(
                    out=ab[:, 0:main],
                    in_=tfall[:, g0 : g0 + main],
                    func=mybir.ActivationFunctionType.Abs,
                    bias=bvec[:, c : c + 1],
                )
                nc.scalar.activation(
                    out=g_blocks[:, c - 1, 0:n_full, 0:127],
                    in_=ab[:, 0:main].rearrange("p (s w) -> p s w", w=127),
                    func=mybir.ActivationFunctionType.Relu,
                    bias=bvec[:, 0:1],
                    scale=-1.0,
                )
            g_tails = g_blocks[:, :, tail_slot, 0:tail]
            tf_tail_b = (
                tfall[:, g0 + main : g0 + fk].unsqueeze(1).broadcast_to([P, 7, tail])
            )
            cvec_b = cvec[:, :].unsqueeze(2).broadcast_to([P, 7, tail])
            nc.vector.tensor_tensor(
                g_tails, tf_tail_b, cvec_b, op=mybir.AluOpType.is_equal
            )

        emit_gt(0)

        nchunks = len(CHUNKS)
        for k, (pieces, fk, tail_slot) in enumerate(CHUNKS):
            n_full = fk // 127          # full 127-px subtiles
            tail = fk - 127 * n_full    # tail pixels (4 or 8)
            main = 127 * n_full
            subtiles = list(range(n_full)) + [tail_slot]

            # ---- load: one 8-channel cast DMA per piece (f32 -> fp16),
            # classes laid at stride FMAX inside chall ----
            chall = chpool.tile([P, C * FMAX], mybir.dt.float16, name="chall", tag="chall")
            dst = 0
            for (n, off, ln) in pieces:
                ch_dst = chall[:, :].rearrange("p (c w) -> p c w", c=C)[
                    :, :, dst : dst + ln
                ]
                nc.gpsimd.dma_start(
                    out=ch_dst,
                    in_=yp[n][:, :, off : off + ln].rearrange("c p w -> p c w"),
                )
                dst += ln

            ch = [chall[:, c * FMAX : c * FMAX + fk] for c in range(C)]

            # ---- max tree (DVE, fp16 tensor_tensor => 2x mode) ----
            m01 = mtmp.tile([P, FMAX], mybir.dt.float16, name="m01", tag="m01")
            nc.vector.tensor_max(m01[:, 0:fk], ch[0], ch[1])
            m23 = mtmp.tile([P, FMAX], mybir.dt.float16, name="m23", tag="m23")
            nc.vector.tensor_max(m23[:, 0:fk], ch[2], ch[3])
            m45 = mtmp.tile([P, FMAX], mybir.dt.float16, name="m45", tag="m45")
            nc.vector.tensor_max(m45[:, 0:fk], ch[4], ch[5])
            m67 = mtmp.tile([P, FMAX], mybir.dt.float16, name="m67", tag="m67")
            nc.vector.tensor_max(m67[:, 0:fk], ch[6], ch[7])
            m0123 = mtmp.tile([P, FMAX], mybir.dt.float16, name="m0123", tag="m0123")
            nc.vector.tensor_max(m0123[:, 0:fk], m01[:, 0:fk], m23[:, 0:fk])
            m4567 = mtmp.tile([P, FMAX], mybir.dt.float16, name="m4567", tag="m4567")
            nc.vector.tensor_max(m4567[:, 0:fk], m45[:, 0:fk], m67[:, 0:fk])
            m = mpool.tile([P, FMAX], mybir.dt.float16, name="m", tag="m")
            nc.vector.tensor_max(m[:, 0:fk], m0123[:, 0:fk], m4567[:, 0:fk])

            p_blocks = Pm[:, :].rearrange("p (c s w) -> p c s w", c=7, w=128)

            # ---- pred masks ----
            # batched tail compare (all 7 classes, 1 op)
            p_tails = p_blocks[:, :, tail_slot, 0:tail]
            ch_tails = chall[:, :].rearrange("p (c w) -> p c w", c=C)[
                :, 1:C, main : main + tail
            ]
            m_tail_b = m[:, main:fk].unsqueeze(1).broadcast_to([P, 7, tail])
            nc.vector.tensor_tensor(
                p_tails, ch_tails, m_tail_b, op=mybir.AluOpType.is_equal
            )
            # batched main compare: all 7 classes in one 4D-AP tensor_tensor
            p_main = p_blocks[:, :, 0:n_full, 0:127]
            ch_main = chall[:, FMAX : C * FMAX].rearrange(
                "p (c w) -> p c w", c=7
            )[:, :, 0:main].rearrange("p c (s w) -> p c s w", w=127)
            m_main_b = (
                m[:, 0:main]
                .rearrange("p (s w) -> p s w", w=127)
                .unsqueeze(1)
                .broadcast_to([P, 7, n_full, 127])
            )
            nc.vector.tensor_tensor(
                p_main, ch_main, m_main_b, op=mybir.AluOpType.is_equal
            )

            # ---- prefetch next chunk's gt masks (fills DVE while this
            # chunk's matmuls run) ----
            if k + 1 < nchunks:
                emit_gt(k + 1)

            # ---- PE: one N=128 matmul per (class, subtile) ----
            Gm = G01[k % 2]
            for c in range(1, C):
                blk = (c - 1) * BW
                for i, s in enumerate(subtiles):
                    nc.tensor.matmul(
                        psums[c - 1],
                        lhsT=Pm[:, blk + s * 128 : blk + (s + 1) * 128],
                        rhs=Gm[:, blk + s * 128 : blk + (s + 1) * 128],
                        start=(k == 0 and i == 0),
                        stop=(k == nchunks - 1 and i == len(subtiles) - 1),
                    )

        for c in range(7):
            tps = drainp.tile([P, 128], mybir.dt.float32, name=f"tps{c}", tag=f"tps{c}")
            nc.scalar.copy(out=tps, in_=psums[c])
            nc.sync.dma_start(out=tp_out[c], in_=tps)

    nc.finalize()
    return nc


def _get_bass():
    global _CACHED_NC
    if _CACHED_NC is None:
        _CACHED_NC = build_bass()
    return _CACHED_NC


def make_in_maps(y_true, y_pred):
    yp = np.ascontiguousarray(np.asarray(y_pred, dtype=np.float32))
    # labels are 0..7: uint8 re-encoding is lossless
    yt = np.asarray(y_true).astype(np.uint8)
    in_maps = []
    for i in range(N_CORES):
        yps = np.ascontiguousarray(yp[NB * i : NB * (i + 1)]).reshape(NB, C, P, FP)
        yts = np.ascontiguousarray(yt[NB * i : NB * (i + 1)]).reshape(NB, P, FP)
        in_maps.append({"yp": yps, "yt": yts})
    return in_maps


def epilogue(results):
    """Combine the 8 cores' partial sums into the final dice mean (float32,
    mirroring the reference arithmetic)."""
    tp = np.zeros(7, dtype=np.float64)
    pred_cnt = np.zeros(7, dtype=np.float64)
    gt_cnt = np.zeros(7, dtype=np.float64)
    for r in results:
        po = np.asarray(r["tp_out"], dtype=np.float64)  # [7, 128, 128]
        tp += np.trace(po[:, 0:127, 0:127], axis1=1, axis2=2)
        pred_cnt += po[:, 0:127, 127].sum(axis=1)
        gt_cnt += po[:, 127, 0:127].sum(axis=1)

    tp32 = tp.astype(np.float32)
    fp32_ = (pred_cnt - tp).astype(np.float32)
    fn32 = (gt_cnt - tp).astype(np.float32)
    eps = np.float32(EPS)
    two = np.float32(2.0)
    dice = (two * tp32 + eps) / (two * tp32 + fp32_ + fn32 + eps)
    return np.asarray(np.mean(dice, dtype=np.float32), dtype=np.float32)


def kernel(**inputs):
    from concourse.bass_utils import run_bass_kernel_spmd

    nc = _get_bass()
    in_maps = make_in_maps(inputs["y_true"], inputs["y_pred"])
    res = run_bass_kernel_spmd(nc, in_maps, core_ids=list(range(N_CORES)))
    return epilogue(res.results)


if __name__ == "__main__":
    # smoke test with random data
    rng = np.random.default_rng(0)
    y_true = rng.integers(0, C, size=(16, 512, 512)).astype(np.int32)
    y_pred = rng.standard_normal((16, C, 512, 512)).astype(np.float32)
    out = kernel(y_true=y_true, y_pred=y_pred)
    print("kernel output:", out)


# revision 16
# speedup vs baseline: 1.2775x; 1.2775x over previous
"""Trainium2 Bass kernel for DiceLoss (hard-argmax dice, ignore background, mean).

Problem (hardcoded shapes):
  y_true: [16, 512, 512] int32 in [0, 8)
  y_pred: [16, 8, 512, 512] float32
  out   : scalar float32 = mean over classes 1..7 of
          (2*tp + eps) / (2*tp + fp + fn + eps)
  with pred_cls = argmax_c y_pred, one-hot tp/fp/fn sums over all pixels.

Strategy (8 NeuronCores, data-parallel over batch):
  - Each core processes 2 of the 16 batch images (SPMD, same NEFF), streamed
    as 5 chunks of [512, 1024, 1024, 1024, 512] pixel-columns (the middle
    chunk spans the image boundary -- pixels are pixels for global tp/fp/fn
    sums; the small tail chunk shortens the end-of-stream compute that
    cannot overlap DMA).
  - y_pred is cast f32 -> fp16 during the DMA itself (SWDGE CME cast, one
    8-channel DMA per chunk; HBM read traffic unchanged).  fp16
    equality-vs-max introduces spurious argmax ties at ~5e-4 of pixels ->
    rel err ~2e-4 on the final dice (tolerance 2e-2).
  - Labels are staged as uint8 (lossless re-encoding of values 0..7),
    loaded upfront via HWDGE and converted once to fp16 on the
    otherwise-idle ScalarE.  Each chunk's gt masks are computed one chunk
    AHEAD (double-buffered G tile), so the per-chunk DVE work on the
    critical path (max tree + pred masks) stays under the chunk's DMA
    time and the final chunk leaves only ~4us of compute after the last
    HBM byte.
  - DVE (VectorE), all ops in measured fast perf modes (accum_out is never
    used: it drops DVE to 1x mode on HW; scalar_tensor_tensor is 1x-only):
      * 7-op pairwise tensor_tensor MAX tree (fp16, 2x)
      * pred masks (ch[c] == m): one batched 4D-AP tensor_tensor IS_EQUAL
        over all 7 classes (fp16, 2x)
      * gt masks (labels == c): 7 tensor_scalar IS_EQUAL (fp16, 4x)
      * 2 batched tail-compare ops per chunk
  - Mask layout: per class, 10 subtiles of 128 columns = [127 px | 1 ones
    col]; a 1024-px chunk = 8 full subtiles + an 8-px tail in s=8, a
    512-px chunk = 4 full + a 4-px tail (s=4 for the head chunk, virgin
    s=9 for the last chunk so no pad re-zeroing is ever needed).
  - TensorE: per (class, subtile) one matmul psum_c += P_cs^T @ G_cs
    (N=128) accumulated over all subtiles/chunks.  In the [128,128] psum:
    diag[0:127] = tp, col 127 = per-col pred counts, row 127 = per-col gt
    counts -- all three statistics from the same matmul stream.  One psum
    bank per class so the final drains overlap the tail chunk's compute.
  - Host: sums the 8 cores' exact-integer partials and forms the dice mean
    in float32, mirroring the reference arithmetic.
"""

import numpy as np

EPS = 1e-05

# Problem geometry (hardcoded per the harness contract).
N_CORES = 8
NB = 2             # batch images per core
C = 8              # classes
P = 128            # SBUF partitions
FP = 2048          # free-dim elements per image plane ([128, 2048] = 512*512)
FT = NB * FP       # label columns per core (4096)
FMAX = 1024        # max pixels per chunk (class stride inside chall)
NSUBT = 10         # subtiles per class block (incl. virgin tail slot s=9)
BW = NSUBT * 128   # mask-tile columns per class block (1280)

# Chunk schedule: (pieces, px, tail_slot); pieces = [(img, off, len), ...].
# Global label offset of a chunk = img*FP + off of its first piece.
CHUNKS = [
    ([(0, 0, 512)], 512, 4),
    ([(0, 512, 1024)], 1024, 8),
    ([(0, 1536, 512), (1, 0, 512)], 1024, 8),
    ([(1, 512, 1024)], 1024, 8),
    ([(1, 1536, 512)], 512, 9),
]

_CACHED_NC = None


def build_bass():
    """Build the Bass kernel (same NEFF for all 8 cores)."""
    from contextlib import ExitStack

    import concourse.bacc as bacc
    import concourse.tile as tile
    from concourse import mybir

    nc = bacc.Bacc(None, target_bir_lowering=False)

    yp = nc.dram_tensor("yp", [NB, C, P, FP], mybir.dt.float32, kind="ExternalInput")
    yt = nc.dram_tensor("yt", [NB, P, FP], mybir.dt.uint8, kind="ExternalInput")
    # per class: [128, 128] psum (diag = tp, col 127 = pred cnt, row 127 = gt cnt)
    tp_out = nc.dram_tensor("tp_out", [7, P, 128], mybir.dt.float32, kind="ExternalOutput")

    with tile.TileContext(nc) as tc, ExitStack() as ctx:
        chpool = ctx.enter_context(tc.tile_pool(name="ch", bufs=3))
        mpool = ctx.enter_context(tc.tile_pool(name="mx", bufs=2))
        mtmp = ctx.enter_context(tc.tile_pool(name="mtmp", bufs=2))
        maskp = ctx.enter_context(tc.tile_pool(name="mask", bufs=1))
        abspool = ctx.enter_context(tc.tile_pool(name="ab", bufs=2))
        drainp = ctx.enter_context(tc.tile_pool(name="drain", bufs=1))
        psump = ctx.enter_context(tc.tile_pool(name="psum", bufs=1, space="PSUM"))

        # Persistent tiles.  G is manually double-buffered (G0/G1) so chunk
        # k+1's gt masks can be built while chunk k's matmuls read G[k%2].
        Pm = maskp.tile([P, 7 * BW], mybir.dt.float16, name="Pm", tag="Pm")
        G01 = [
            maskp.tile([P, 7 * BW], mybir.dt.float16, name=f"G{i}", tag=f"G{i}")
            for i in range(2)
        ]
        # cvec[:, i] = i+1 (class constants for the batched gt tail compare)
        cvec = maskp.tile([P, 7], mybir.dt.float16, name="cvec", tag="cvec")
        # all labels upfront: uint8 via HWDGE, one fp16 convert on ScalarE
        t8 = maskp.tile([P, FT], mybir.dt.uint8, name="t8", tag="t8")
        tfall = maskp.tile([P, FT], mybir.dt.float16, name="tfall", tag="tfall")
        for n in range(NB):
            nc.sync.dma_start(out=t8[:, n * FP : (n + 1) * FP], in_=yt[n])
        nc.scalar.copy(out=tfall, in_=t8)

        # One-time init (column-targeted; a full-tile memset costs ~7us of
        # DVE and would delay the first chunk's compute):
        #   - ones column (col 127 of each subtile block)
        #   - zero pads of the tail subtiles (s=4/s=9 cols[4:127], s=8
        #     cols[8:127]); mask ops only ever write px columns, so pads
        #     stay 0 and ones columns stay 1 across all chunks.
        # ones cols: write a 16-wide stripe [112:128] (fast mode; cols
        # 112..126 of full subtiles get overwritten by px writes, and the
        # pad memsets below re-zero them inside tail subtiles)
        for t in (Pm, G01[0], G01[1]):
            blocks = t[:, :].rearrange("p (c s w) -> p c s w", c=7, w=128)
            nc.vector.memset(blocks[:, :, :, 112:128], 1.0)
            nc.vector.memset(blocks[:, :, 4, 4:127], 0.0)
            nc.vector.memset(blocks[:, :, 8, 8:127], 0.0)
            nc.vector.memset(blocks[:, :, 9, 4:127], 0.0)
        for i in range(7):
            nc.vector.memset(cvec[:, i : i + 1], float(i + 1))
        # bias constants for the ACT gt-mask ops: col 0 = +1.0 (relu bias),
        # col c = -c (abs bias)
        bvec = maskp.tile([P, 8], mybir.dt.float16, name="bvec", tag="bvec")
        nc.vector.memset(bvec[:, 0:1], 1.0)
        for c in range(1, 8):
            nc.vector.memset(bvec[:, c : c + 1], -float(c))

        # One psum bank per class: drains only wait on their own chain.
        psums = [
            psump.tile([P, 128], mybir.dt.float32, name=f"ps{c}", tag=f"ps{c}")
            for c in range(7)
        ]

        def emit_gt(k):
            """gt masks for chunk k.  Main masks run on the otherwise-idle
            ScalarE as relu(1 - |label - c|) -- exact for integer labels in
            fp16 -- keeping DVE free for the max tree + pred compares.  The
            tiny tail compare stays on DVE (1 batched op)."""
            pieces, fk, tail_slot = CHUNKS[k]
            n_full = fk // 127
            tail = fk - 127 * n_full
            main = 127 * n_full
            g0 = pieces[0][0] * FP + pieces[0][1]  # global label offset
            g_blocks = G01[k % 2][:, :].rearrange("p (c s w) -> p c s w", c=7, w=128)
            tf_main = tfall[:, g0 : g0 + main].rearrange("p (s w) -> p s w", w=127)
            for c in range(1, 4):
                # classes 1..3 on ScalarE (relu(1-|label-c|), exact for ints)
                ab = abspool.tile([P, FMAX], mybir.dt.float16, name="ab", tag="ab")
                nc.scalar.activation(
                    out=ab[:, 0:main],
                    in_=tfall[:, g0 : g0 + main],
                    func=mybir.ActivationFunctionType.Abs,
                    bias=bvec[:, c : c + 1],
                )
                nc.scalar.activation(
                    out=g_blocks[:, c - 1, 0:n_full, 0:127],
                    in_=ab[:, 0:main].rearrange("p (s w) -> p s w", w=127),
                    func=mybir.ActivationFunctionType.Relu,
                    bias=bvec[:, 0:1],
                    scale=-1.0,
                )
            for c in range(4, C):
                # classes 4..7 on DVE (tensor_scalar is_equal, 4x mode)
                nc.vector.tensor_scalar(
                    out=g_blocks[:, c - 1, 0:n_full, 0:127],
                    in0=tf_main,
                    scalar1=float(c),
                    scalar2=0.0,
                    op0=mybir.AluOpType.is_equal,
                    op1=mybir.AluOpType.add,
                )
            g_tails = g_blocks[:, :, tail_slot, 0:tail]
            tf_tail_b = (
                tfall[:, g0 + main : g0 + fk].unsqueeze(1).broadcast_to([P, 7, tail])
            )
            cvec_b = cvec[:, :].unsqueeze(2).broadcast_to([P, 7, tail])
            nc.vector.tensor_tensor(
                g_tails, tf_tail_b, cvec_b, op=mybir.AluOpType.is_equal
            )

        emit_gt(0)

        nchunks = len(CHUNKS)
        for k, (pieces, fk, tail_slot) in enumerate(CHUNKS):
            n_full = fk // 127          # full 127-px subtiles
            tail = fk - 127 * n_full    # tail pixels (4 or 8)
            main = 127 * n_full
            subtiles = list(range(n_full)) + [tail_slot]

            # ---- load: one 8-channel cast DMA per piece (f32 -> fp16),
            # classes laid at stride FMAX inside chall ----
            chall = chpool.tile([P, C * FMAX], mybir.dt.float16, name="chall", tag="chall")
            dst = 0
            for (n, off, ln) in pieces:
                ch_dst = chall[:, :].rearrange("p (c w) -> p c w", c=C)[
                    :, :, dst : dst + ln
                ]
                nc.gpsimd.dma_start(
                    out=ch_dst,
                    in_=yp[n][:, :, off : off + ln].rearrange("c p w -> p c w"),
                )
                dst += ln

            ch = [chall[:, c * FMAX : c * FMAX + fk] for c in range(C)]

            # ---- max tree (DVE, fp16 tensor_tensor => 2x mode) ----
            m01 = mtmp.tile([P, FMAX], mybir.dt.float16, name="m01", tag="m01")
            nc.vector.tensor_max(m01[:, 0:fk], ch[0], ch[1])
            m23 = mtmp.tile([P, FMAX], mybir.dt.float16, name="m23", tag="m23")
            nc.vector.tensor_max(m23[:, 0:fk], ch[2], ch[3])
            m45 = mtmp.tile([P, FMAX], mybir.dt.float16, name="m45", tag="m45")
            nc.vector.tensor_max(m45[:, 0:fk], ch[4], ch[5])
            m67 = mtmp.tile([P, FMAX], mybir.dt.float16, name="m67", tag="m67")
            nc.vector.tensor_max(m67[:, 0:fk], ch[6], ch[7])
            m0123 = mtmp.tile([P, FMAX], mybir.dt.float16, name="m0123", tag="m0123")
            nc.vector.tensor_max(m0123[:, 0:fk], m01[:, 0:fk], m23[:, 0:fk])
            m4567 = mtmp.tile([P, FMAX], mybir.dt.float16, name="m4567", tag="m4567")
            nc.vector.tensor_max(m4567[:, 0:fk], m45[:, 0:fk], m67[:, 0:fk])
            m = mpool.tile([P, FMAX], mybir.dt.float16, name="m", tag="m")
            nc.vector.tensor_max(m[:, 0:fk], m0123[:, 0:fk], m4567[:, 0:fk])

            p_blocks = Pm[:, :].rearrange("p (c s w) -> p c s w", c=7, w=128)

            # ---- pred masks ----
            # batched tail compare (all 7 classes, 1 op)
            p_tails = p_blocks[:, :, tail_slot, 0:tail]
            ch_tails = chall[:, :].rearrange("p (c w) -> p c w", c=C)[
                :, 1:C, main : main + tail
            ]
            m_tail_b = m[:, main:fk].unsqueeze(1).broadcast_to([P, 7, tail])
            nc.vector.tensor_tensor(
                p_tails, ch_tails, m_tail_b, op=mybir.AluOpType.is_equal
            )
            # batched main compare: all 7 classes in one 4D-AP tensor_tensor
            p_main = p_blocks[:, :, 0:n_full, 0:127]
            ch_main = chall[:, FMAX : C * FMAX].rearrange(
                "p (c w) -> p c w", c=7
            )[:, :, 0:main].rearrange("p c (s w) -> p c s w", w=127)
            m_main_b = (
                m[:, 0:main]
                .rearrange("p (s w) -> p s w", w=127)
                .unsqueeze(1)
                .broadcast_to([P, 7, n_full, 127])
            )
            nc.vector.tensor_tensor(
                p_main, ch_main, m_main_b, op=mybir.AluOpType.is_equal
            )

            # ---- prefetch next chunk's gt masks (fills DVE while this
            # chunk's matmuls run) ----
            if k + 1 < nchunks:
                emit_gt(k + 1)

            # ---- PE: one N=128 matmul per (class, subtile) ----
            Gm = G01[k % 2]
            for c in range(1, C):
                blk = (c - 1) * BW
                for i, s in enumerate(subtiles):
                    nc.tensor.matmul(
                        psums[c - 1],
                        lhsT=Pm[:, blk + s * 128 : blk + (s + 1) * 128],
                        rhs=Gm[:, blk + s * 128 : blk + (s + 1) * 128],
                        start=(k == 0 and i == 0),
                        stop=(k == nchunks - 1 and i == len(subtiles) - 1),
                    )

        for c in range(7):
            tps = drainp.tile([P, 128], mybir.dt.float32, name=f"tps{c}", tag=f"tps{c}")
            nc.scalar.copy(out=tps, in_=psums[c])
            nc.sync.dma_start(out=tp_out[c], in_=tps)

    nc.finalize()
    return nc


def _get_bass():
    global _CACHED_NC
    if _CACHED_NC is None:
        _CACHED_NC = build_bass()
    return _CACHED_NC


def make_in_maps(y_true, y_pred):
    yp = np.ascontiguousarray(np.asarray(y_pred, dtype=np.float32))
    # labels are 0..7: uint8 re-encoding is lossless
    yt = np.asarray(y_true).astype(np.uint8)
    in_maps = []
    for i in range(N_CORES):
        yps = np.ascontiguousarray(yp[NB * i : NB * (i + 1)]).reshape(NB, C, P, FP)
        yts = np.ascontiguousarray(yt[NB * i : NB * (i + 1)]).reshape(NB, P, FP)
        in_maps.append({"yp": yps, "yt": yts})
    return in_maps


def epilogue(results):
    """Combine the 8 cores' partial sums into the final dice mean (float32,
    mirroring the reference arithmetic)."""
    tp = np.zeros(7, dtype=np.float64)
    pred_cnt = np.zeros(7, dtype=np.float64)
    gt_cnt = np.zeros(7, dtype=np.float64)
    for r in results:
        po = np.asarray(r["tp_out"], dtype=np.float64)  # [7, 128, 128]
        tp += np.trace(po[:, 0:127, 0:127], axis1=1, axis2=2)
        pred_cnt += po[:, 0:127, 127].sum(axis=1)
        gt_cnt += po[:, 127, 0:127].sum(axis=1)

    tp32 = tp.astype(np.float32)
    fp32_ = (pred_cnt - tp).astype(np.float32)
    fn32 = (gt_cnt - tp).astype(np.float32)
    eps = np.float32(EPS)
    two = np.float32(2.0)
    dice = (two * tp32 + eps) / (two * tp32 + fp32_ + fn32 + eps)
    return np.asarray(np.mean(dice, dtype=np.float32), dtype=np.float32)


def kernel(**inputs):
    from concourse.bass_utils import run_bass_kernel_spmd

    nc = _get_bass()
    in_maps = make_in_maps(inputs["y_true"], inputs["y_pred"])
    res = run_bass_kernel_spmd(nc, in_maps, core_ids=list(range(N_CORES)))
    return epilogue(res.results)


if __name__ == "__main__":
    # smoke test with random data
    rng = np.random.default_rng(0)
    y_true = rng.integers(0, C, size=(16, 512, 512)).astype(np.int32)
    y_pred = rng.standard_normal((16, C, 512, 512)).astype(np.float32)
    out = kernel(y_true=y_true, y_pred=y_pred)
    print("kernel output:", out)
